# revision 9
# baseline (speedup 1.0000x reference)
"""Trainium2 Bass kernel: batched multi-head cross-attention.

Reference computation (per batch element b):
    q = x @ Wq; k,v = split(context @ Wkv)
    per head: attn = softmax(q k^T / 8); o = attn @ v
    out = concat_heads(o) @ Wo + bo

Sharding: pure data parallel - batch B=8, one batch element per NeuronCore,
no collectives. Fully "transposed" dataflow (host feeds x^T / context^T,
host transposes outT back after gather):

    QT[d,i]  = Wq^T  @ xT
    KT[d,j]  = Wk^T  @ cT
    V[j,d]   = cT^T  @ Wv
    ST[j,i]  = KT_h^T @ QT_h        per head, contraction d=64. The two heads
                                    of a pair live in disjoint 64-partition
                                    halves of kt/qt, so their score matmuls
                                    auto-derive tile_position (0,0)/(64,0)
                                    and run CONCURRENTLY on the PE array
                                    (row-group tiling) - 2x score throughput.
    PT       = exp(ST / 8)          (no max-subtraction: scores are O(6) so
                                     exp is safe in fp32, and softmax is
                                     shift-invariant => exact same result)
    OunT,l   = [V_h | 1x64]^T @ PT  M=128 stationary: rows 0-63 get the
                                    unnormalized output, rows 64-127 all get
                                    the softmax denominator (64 identical
                                    ones-columns) - the denominator arrives
                                    ALREADY BROADCAST across 64 partitions
                                    for the same cycle cost as M=65.
    OnT      = OunT[0:64] * recip(OunT[64:128])   ([64,N] DVE ops only)
    outT     = Wo^T @ OnT + bo

Schedule: the exp stream on the Scalar engine is the critical path
(~64K+ lanes-cycles). Score tiles are FD=512 (one PSUM bank), so phase C
needs only 2 (scores) + 4 (PV accumulators) banks, leaving a persistent
2-bank rotating pool through which everything else streams DURING the
exp-bound attention loop:
  - prologue: only Q/K for pair 0 + V chunks 0-3
  - pairs 0-1 slots: remaining V chunks, then Q/K for pairs 1-3
  - pair 3 slots: output-projection partials (Wo head-chunks 0-2 + bias)
  - tail: per output group one final matmul (head-chunk 3) + one add.
Matmul inputs are bf16 (full-rate PE), accumulation fp32 in PSUM.
"""

import numpy as np
import ml_dtypes

B, N, M, D = 8, 1024, 1024, 512
H, DH = 8, 64
KC = 4
IC = 2
JC = 8
NP = 4
N_CORES = 8

_BF16 = ml_dtypes.bfloat16
_CACHE = {}
LAST_RUN = None


def _build_nc():
    import concourse.bass as bass
    import concourse.mybir as mybir
    import concourse.tile as tile
    from concourse import bacc

    f32 = mybir.dt.float32
    bf16 = mybir.dt.bfloat16
    Exp = mybir.ActivationFunctionType.Exp

    nc = bacc.Bacc()

    xt = nc.declare_dram_parameter("xt", [D, N], bf16, isOutput=False)
    ct = nc.declare_dram_parameter("ct", [D, M], bf16, isOutput=False)
    wq = nc.declare_dram_parameter("wq", [D, D], bf16, isOutput=False)
    wk = nc.declare_dram_parameter("wk", [D, D], bf16, isOutput=False)
    wv = nc.declare_dram_parameter("wv", [D, D], bf16, isOutput=False)
    wo = nc.declare_dram_parameter("wo", [D, D], bf16, isOutput=False)
    bo = nc.declare_dram_parameter("bo", [D, 1], f32, isOutput=False)
    outT = nc.declare_dram_parameter("outT", [D, N], f32, isOutput=True)

    with tile.TileContext(nc) as tc:
        with (
            tc.tile_pool(name="singles", bufs=1) as singles,
            tc.tile_pool(name="pt", bufs=4) as ptp,
            tc.tile_pool(name="linv", bufs=4) as linvp,
            tc.tile_pool(name="pout", bufs=3) as poutp,
        ):
            def sb(shape, dt, tag):
                return singles.tile(shape, dt, tag=tag, name=tag)

            wq_sb = [sb([128, D], bf16, f"wq{c}") for c in range(KC)]
            xt_sb = [sb([128, N], bf16, f"xt{c}") for c in range(KC)]
            wk_sb = [sb([128, D], bf16, f"wk{c}") for c in range(KC)]
            ct_sb = [sb([128, M], bf16, f"ct{c}") for c in range(KC)]
            wv_sb = [sb([128, D], bf16, f"wv{c}") for c in range(KC)]
            wo_sb = [sb([128, D], bf16, f"wo{c}") for c in range(KC)]
            bo_sb = sb([128, KC, 1], f32, "bo")

            # Loads: Q deps (wq head + xt) on one HWDGE queue, K/V deps
            # (wk head + ct) on the other, bulky late weights via SWDGE.
            nc.sync.dma_start(out=wq_sb[0], in_=wq[0:128, :])
            nc.scalar.dma_start(out=wk_sb[0], in_=wk[0:128, :])
            for c in range(KC):
                nc.sync.dma_start(out=xt_sb[c], in_=xt[c * 128:(c + 1) * 128, :])
                nc.scalar.dma_start(out=ct_sb[c], in_=ct[c * 128:(c + 1) * 128, :])
            for c in range(1, KC):
                nc.sync.dma_start(out=wq_sb[c], in_=wq[c * 128:(c + 1) * 128, :])
                nc.scalar.dma_start(out=wk_sb[c], in_=wk[c * 128:(c + 1) * 128, :])
            for c in range(KC):
                nc.gpsimd.dma_start(out=wv_sb[c], in_=wv[c * 128:(c + 1) * 128, :])
            for c in range(KC):
                nc.gpsimd.dma_start(out=wo_sb[c], in_=wo[c * 128:(c + 1) * 128, :])
            nc.gpsimd.dma_start(
                out=bo_sb, in_=bo[:, :].rearrange("(c p) o -> p c o", p=128)
            )

            qt_sb = [sb([128, N], bf16, f"qt{c}") for c in range(KC)]
            kt_sb = [sb([128, M], bf16, f"kt{c}") for c in range(KC)]
            vv_sb = [sb([128, H, 2 * DH], bf16, f"vv{j}") for j in range(JC)]
            on_sb = [sb([128, N], bf16, f"on{c}") for c in range(KC)]
            # fp32 partial of the output projection (Wo head-chunks 0-2 +
            # bias), produced during pair 3, consumed in the tail.
            part_sb = [sb([128, 512], f32, f"part{g}") for g in range(8)]

            with (
                tc.tile_pool(name="psO", bufs=2, space="PSUM") as psO,
                tc.tile_pool(name="rot", bufs=2, space="PSUM") as rot,
                tc.tile_pool(name="psS", bufs=2, space="PSUM") as psS,
            ):
                def emit_qk_half(dst_tiles, w_tiles, src_tiles, dc, icc):
                    ps = rot.tile([128, 512], f32, tag="rot", name="rot")
                    for kcc in range(KC):
                        nc.tensor.matmul(
                            ps,
                            lhsT=w_tiles[kcc][:, dc * 128:(dc + 1) * 128],
                            rhs=src_tiles[kcc][:, icc * 512:(icc + 1) * 512],
                            start=(kcc == 0),
                            stop=(kcc == KC - 1),
                        )
                    nc.vector.tensor_copy(
                        dst_tiles[dc][:, icc * 512:(icc + 1) * 512], ps
                    )

                def emit_v(jc):
                    ps = rot.tile([128, 512], f32, tag="rot", name="rot")
                    for kcc in range(KC):
                        nc.tensor.matmul(
                            ps,
                            lhsT=ct_sb[kcc][:, jc * 128:(jc + 1) * 128],
                            rhs=wv_sb[kcc],
                            start=(kcc == 0),
                            stop=(kcc == KC - 1),
                        )
                    nc.vector.memset(vv_sb[jc][:, :, DH:2 * DH], 1.0)
                    nc.vector.tensor_copy(
                        vv_sb[jc][:, :, 0:DH],
                        ps.rearrange("p (h d) -> p h d", h=H),
                    )

                def emit_opart(g):
                    ec, icc = g // 2, g % 2
                    ps = rot.tile([128, 512], f32, tag="rot", name="rot")
                    for hc in range(3):
                        nc.tensor.matmul(
                            ps,
                            lhsT=wo_sb[hc][:, ec * 128:(ec + 1) * 128],
                            rhs=on_sb[hc][:, icc * 512:(icc + 1) * 512],
                            start=(hc == 0),
                            stop=(hc == 2),
                        )
                    nc.vector.tensor_scalar_add(part_sb[g], ps, bo_sb[:, ec, :])

                # prologue projections: pair-0 deps only
                for icc in range(IC):
                    emit_qk_half(qt_sb, wq_sb, xt_sb, 0, icc)
                for icc in range(IC):
                    emit_qk_half(kt_sb, wk_sb, ct_sb, 0, icc)
                for jc in range(4):
                    emit_v(jc)

                chunks = []
                for jc in range(4, JC):
                    chunks.append(lambda jc=jc: emit_v(jc))
                for dc in range(1, KC):
                    for icc in range(IC):
                        chunks.append(
                            lambda dc=dc, icc=icc: emit_qk_half(qt_sb, wq_sb, xt_sb, dc, icc)
                        )
                        chunks.append(
                            lambda dc=dc, icc=icc: emit_qk_half(kt_sb, wk_sb, ct_sb, dc, icc)
                        )
                # pair 2 slots: idle; pair 3 slots: output-projection partials
                chunks += [None] * 8
                chunks += [lambda g=g: emit_opart(g) for g in range(8)]
                ci = 0

                for p in range(NP):
                    pso = [
                        psO.tile([128, N], f32, tag="psO", name="psO")
                        for _ in range(2)
                    ]
                    for jc in range(JC):
                        ptiles = [
                            ptp.tile([128, N], bf16, tag="pt", name="pt")
                            for _ in range(2)
                        ]
                        for icc in range(IC):
                            pss = [
                                psS.tile([128, 512], f32, tag="psS", name="psS")
                                for _ in range(2)
                            ]
                            for hh in range(2):
                                pb = hh * 64
                                nc.tensor.matmul(
                                    pss[hh],
                                    lhsT=kt_sb[p][pb:pb + 64, jc * 128:(jc + 1) * 128],
                                    rhs=qt_sb[p][pb:pb + 64, icc * 512:(icc + 1) * 512],
                                    start=True,
                                    stop=True,
                                )
                            for hh in range(2):
                                h = 2 * p + hh
                                nc.scalar.activation(
                                    out=ptiles[hh][:, icc * 512:(icc + 1) * 512],
                                    in_=pss[hh], func=Exp, scale=0.125,
                                )
                                nc.tensor.matmul(
                                    pso[hh][:, icc * 512:(icc + 1) * 512],
                                    lhsT=vv_sb[jc][:, h, :],
                                    rhs=ptiles[hh][:, icc * 512:(icc + 1) * 512],
                                    start=(jc == 0),
                                    stop=(jc == JC - 1),
                                )
                        if ci < len(chunks):
                            if chunks[ci] is not None:
                                chunks[ci]()
                            ci += 1
                    # normalization: rows 64:127 of pso hold the softmax
                    # denominator already broadcast across 64 partitions.
                    # reciprocal_approx_fast misreads PSUM sources at
                    # base-partition 64 (HW-verified), so stage the
                    # denominator into SBUF with a plain copy first.
                    for hh in range(2):
                        lden = linvp.tile([64, N], f32, tag="lden", name="lden")
                        linv = linvp.tile([64, N], f32, tag="linv", name="linv")
                        nc.vector.tensor_copy(lden, pso[hh][64:128, :])
                        nc.vector.reciprocal_approx_fast(out=linv, in_=lden)
                        nc.vector.tensor_mul(
                            on_sb[p][hh * 64:(hh + 1) * 64, :],
                            pso[hh][0:64, :],
                            linv,
                        )

                # tail: per output group, only the head-chunk-3 matmul + add
                for g in range(8):
                    ec, icc = g // 2, g % 2
                    ps = rot.tile([128, 512], f32, tag="rot", name="rot")
                    nc.tensor.matmul(
                        ps,
                        lhsT=wo_sb[3][:, ec * 128:(ec + 1) * 128],
                        rhs=on_sb[3][:, icc * 512:(icc + 1) * 512],
                        start=True,
                        stop=True,
                    )
                    ot = poutp.tile([128, 512], f32, tag="pout", name="pout")
                    nc.vector.tensor_add(ot, ps, part_sb[g])
                    q = nc.sync if (g % 2 == 0) else nc.scalar
                    q.dma_start(
                        out=outT[ec * 128:(ec + 1) * 128, icc * 512:(icc + 1) * 512],
                        in_=ot,
                    )
    nc.finalize()
    return nc


def _ensure_ntff_hook():
    """Install antenv.axon_hooks if the image lacks it, registering the
    ctypes NTFF-profile hook against libaxon_pjrt.so. Without this,
    run_bass_kernel_spmd(trace=True)/BASS_TRACE=1 crashes on import."""
    import contextlib
    import ctypes
    import os
    import sys
    import types

    try:
        import antenv.axon_hooks  # noqa: F401
        return
    except ImportError:
        pass
    try:
        import antenv
    except ImportError:
        return

    state = {"hook": None}
    mod = types.ModuleType("antenv.axon_hooks")
    mod.set_axon_ntff_profile_hook = lambda h: state.__setitem__("hook", h)
    mod.get_axon_ntff_profile_hook = lambda: state["hook"]
    sys.modules["antenv.axon_hooks"] = mod
    antenv.axon_hooks = mod

    so_path = "/opt/axon/libaxon_pjrt.so"
    if not os.path.exists(so_path):
        return
    try:
        lib = ctypes.CDLL(so_path)
    except OSError:
        return
    if not hasattr(lib, "axon_start_nrt_profile"):
        return
    lib.axon_start_nrt_profile.argtypes = [
        ctypes.POINTER(ctypes.c_int64), ctypes.c_size_t,
    ]
    lib.axon_start_nrt_profile.restype = ctypes.c_int64
    lib.axon_stop_nrt_profile.argtypes = [ctypes.c_char_p]
    lib.axon_stop_nrt_profile.restype = ctypes.c_int64

    @contextlib.contextmanager
    def _hook(output_dir, device_ids):
        import jax
        jax.devices()  # force PJRT init so the .so's client exists
        if device_ids:
            ids = (ctypes.c_int64 * len(device_ids))(*device_ids)
            rc = lib.axon_start_nrt_profile(ids, len(device_ids))
        else:
            rc = lib.axon_start_nrt_profile(None, 0)
        if rc != 0:
            raise RuntimeError(f"axon_start_nrt_profile rc={rc}")
        try:
            yield
        finally:
            n = lib.axon_stop_nrt_profile(str(output_dir).encode())
            if n <= 0:
                print(f"ntff profile: rc={n} (no profile output)")

    state["hook"] = _hook


def kernel(x, context, Wq, Wkv, Wo, bo):
    global LAST_RUN
    _ensure_ntff_hook()
    from concourse import bass_utils

    if "nc" not in _CACHE:
        _CACHE["nc"] = _build_nc()
    nc = _CACHE["nc"]

    wq = np.ascontiguousarray(Wq, dtype=np.float32).astype(_BF16)
    wk = np.ascontiguousarray(Wkv[:, :D], dtype=np.float32).astype(_BF16)
    wv = np.ascontiguousarray(Wkv[:, D:], dtype=np.float32).astype(_BF16)
    wo = np.ascontiguousarray(Wo, dtype=np.float32).astype(_BF16)
    bo_ = np.ascontiguousarray(np.asarray(bo, dtype=np.float32).reshape(D, 1))

    in_maps = []
    for b in range(B):
        in_maps.append({
            "xt": np.ascontiguousarray(np.asarray(x[b], np.float32).T).astype(_BF16),
            "ct": np.ascontiguousarray(np.asarray(context[b], np.float32).T).astype(_BF16),
            "wq": wq, "wk": wk, "wv": wv, "wo": wo,
            "bo": bo_,
        })

    LAST_RUN = bass_utils.run_bass_kernel_spmd(nc, in_maps, list(range(N_CORES)))
    out = np.empty((B, N, D), dtype=np.float32)
    for b in range(B):
        out[b] = LAST_RUN.results[b]["outT"].T
    return out
